# revision 19
# baseline (speedup 1.0000x reference)
"""MatchingNet model kernel for 8 Trainium2 NeuronCores — v6 pipelined.

Reference semantics (N=4096, E=512, G=256, V=50000, R=1000):
  x  = embedding[input]          (N, E)
  ex = embedding[set_inputs]     (2, N, E)
  g_out = bidirectional 2-step LSTM over ex   (2, N, E)
  fh = lstm_f(x) + x             (N, E)
  scores[b] = g_out[b] @ fh.T ; a = softmax(scores, axis=0)
  r[b] = a[b] @ g_out[b] ; cosine-reduce over n -> tiny host tail

Sharding: data-parallel over N; core k owns rows [512k, 512k+512).
Attention: a0 = sigmoid((g0-g1) @ fh.T); r1 is never formed — q1 =
a0 @ g1 and the b=1 reductions collapse algebraically on the host.

v6 schedule: a dummy warmup collective absorbs the CC cold-start;
cell order c1 -> f -> c2 -> rcell1 -> rcell2 so fh all-gathers at
~32us and each g half (hf0/hr1/hf1/hr0) is transposed + sent the
moment it is ready, feeding three pipelined all-gathers (fh,
[hf0|hr1], [hf1|hr0]).  Queues: sync HWDGE = small sends, gpsimd =
gathers + collective triggers + big loads, scalar = Act only,
vector = DVE only.  Cell tails run bf16.  Phase D is split into the
half that needs AG-A and the half that needs AG-B; PSUM is retagged
so D1 + both rq halves fit in 8 banks.
"""

import os
import sys

import numpy as np

for _p in ("/opt/trn_rl_repo", os.path.expanduser("~/.axon_site/_ro/trn_rl_repo")):
    if os.path.isdir(_p) and _p not in sys.path:
        sys.path.insert(0, _p)

import concourse.bacc as bacc
import concourse.bass as bass
import concourse.mybir as mybir
import concourse.tile as tile
from concourse import bass_utils
from concourse.masks import make_identity

N, E, G, V, R = 4096, 512, 256, 50000, 1000
NCORES = 8
NL = N // NCORES  # 512 rows per core
P = 128
NE = E // P   # 4 e-chunks
NH = G // P   # 2 hidden chunks for the g-LSTM
NMB = N // P  # 32 m-blocks
EPS = 1e-8

F32 = mybir.dt.float32
BF16 = mybir.dt.bfloat16
F8 = mybir.dt.float8e4
I32 = mybir.dt.int32
AF = mybir.ActivationFunctionType
ALU = mybir.AluOpType
DR = mybir.MatmulPerfMode.DoubleRow
ZG = (0, 2, 3)  # i, g, o (forget gate unused with zero initial state)


def _xgates(nc, pg, xT8, W_sb, gates, hc, s0=0, bias8=None, ones8=None):
    """Stream x-side gate matmuls, moving-tensor-outer for PE locality.

    Returns {gate: psum tile [P, 2, NL]} covering feature chunks
    (g*hc + s0, g*hc + s0 + 1). Bias rides the activation unless bias8
    is given, in which case a rank-1 matmul seeds it into PSUM.
    """
    ps = {g: pg.tile([P, 2, NL], F32, tag="pg2", bufs=3, name="ps_gate")
          for g in gates}
    if bias8 is not None:
        for g in gates:
            for s in range(2):
                jc = g * hc + s0 + s
                nc.tensor.matmul(
                    ps[g][:, s, :], bias8[:, jc * P:(jc + 1) * P],
                    ones8[:], start=True, stop=False,
                    skip_group_check=True)
    for i in range(NE // 2):
        for g in gates:
            for s in range(2):
                jc = g * hc + s0 + s
                js = slice(jc * P, (jc + 1) * P)
                nc.tensor.matmul(
                    ps[g][:, s, :], W_sb[:, 2 * i:2 * i + 2, js],
                    xT8[:, 2 * i:2 * i + 2, :],
                    start=(i == 0 and bias8 is None),
                    stop=(i == NE // 2 - 1),
                    perf_mode=DR, skip_group_check=True)
    return ps


def build_program():
    nc = bacc.Bacc("TRN2", target_bir_lowering=False, debug=False,
                   enable_asserts=False, num_devices=NCORES)
    dram = lambda name, shape, dt=F32, kind="ExternalInput": \
        nc.dram_tensor(name, shape, dt, kind=kind).ap()

    emb8 = dram("emb8", [V, E], F8)
    idx_x = dram("idx_x", [NL, 1], I32)
    idx_e0 = dram("idx_e0", [NL, 1], I32)
    idx_e1 = dram("idx_e1", [NL, 1], I32)
    wgf = dram("wgf", [P, NE, 4 * G], F8)
    wgr = dram("wgr", [P, NE, 4 * G], F8)
    ugf = dram("ugf", [P, NH, 4 * G], F8)
    ugr = dram("ugr", [P, NH, 4 * G], F8)
    wf = dram("wf", [P, NE, 4 * E], F8)
    bgf = dram("bgf", [P, 4 * G // P])
    bgr = dram("bgr", [P, 4 * G // P])
    bf = dram("bf", [P, 4 * E // P])
    bf8 = dram("bf8", [1, 4 * E], F8)
    out = dram("out", [8, E], kind="ExternalOutput")

    with tile.TileContext(nc) as tc:
        _emit(tc, locals())
    nc.compile()
    return nc


def _emit(tc, T):
    nc = tc.nc
    rg = [list(range(NCORES))]
    from contextlib import ExitStack
    ctx = ExitStack()
    with ctx:
        glob = ctx.enter_context(tc.tile_pool(name="glob", bufs=1))
        dramp = ctx.enter_context(tc.tile_pool(name="dramp", bufs=1,
                                               space="DRAM"))

        identf = glob.tile([P, P], F32)
        make_identity(nc, identf)
        ident8 = glob.tile([P, P], F8)
        nc.vector.tensor_copy(out=ident8[:], in_=identf[:])
        ones8 = glob.tile([1, NL], F8)
        nc.gpsimd.memset(ones8[:], 1.0)
        # staging for the 8 per-core reduction rows; one DMA at the end
        stag = glob.tile([P, 8, NE], F32)

        # collective bounce buffers (declared wide: fewer descriptor rows)
        ag1_src_w = dramp.tile([P, 4 * NL], F8)
        ag1_dst_w = dramp.tile([NCORES * P, 4 * NL], F8, addr_space="Shared")
        ag1s = ag1_src_w.rearrange("a (r b) -> (a r) b", r=4)    # (E, NL)
        ag1d = ag1_dst_w.rearrange("a (r b) -> (a r) b", r=4)    # (8E, NL)
        ag2_src_w = dramp.tile([2 * NL // 4, 4 * E], F8)
        ag2_dst_w = dramp.tile([NCORES * 2 * NL // 4, 4 * E], F8,
                               addr_space="Shared")
        ag2s = ag2_src_w.rearrange("a (r b) -> (a r) b", r=4)    # (2NL, E)
        ag2d = ag2_dst_w.rearrange("a (r b) -> (a r) b", r=4)    # (16NL, E)

        # fp8 activations (g_out lives fp8 end-to-end)
        g08 = glob.tile([P, NE, NL], F8)   # [hf0 | hr0]
        g18 = glob.tile([P, NE, NL], F8)   # [hf1 | hr1]
        dgT8 = glob.tile([P, NE, NL], F8)
        A0T = glob.tile([P, NMB, NL], F8)
        fhAll = glob.tile([P, NCORES, NE, NL], F8)   # fh e-major, all cores
        gAll = glob.tile([P, 2 * NMB, E], F8)  # n-major g, block = (k, b, c)


        with tc.tile_pool(name="wpool", bufs=1) as wp, \
             tc.tile_pool(name="acts", bufs=1) as ap_, \
             tc.tile_pool(name="gates", bufs=1) as gp, \
             tc.tile_pool(name="tmp", bufs=1) as tp, \
             tc.tile_pool(name="idxp", bufs=1) as ip, \
             tc.tile_pool(name="raw", bufs=1) as rp, \
             tc.tile_pool(name="pg", bufs=1, space="PSUM") as pgp, \
             tc.tile_pool(name="pt", bufs=1, space="PSUM") as ptp:

            # ---- consolidated idx + weight loads (sync HWDGE) ----
            idxs = {}
            for nm in ("idx_x", "idx_e0", "idx_e1"):
                idxs[nm] = ip.tile([P, NL // P], I32, name=nm + "_sb")
                nc.sync.dma_start(
                    out=idxs[nm][:],
                    in_=T[nm].rearrange("(t p) o -> p (t o)", p=P))
            w_sb = {}
            for nm, kt in (("wf", NE), ("wgf", NE), ("wgr", NE),
                           ("ugf", NH), ("ugr", NH)):
                hw = 4 * (E if nm == "wf" else G)
                w_sb[nm] = wp.tile([P, kt, hw], F8, name=nm + "_sb")
                nc.sync.dma_start(out=w_sb[nm][:], in_=T[nm][:])
            for nm, hw in (("bgf", 8), ("bf", 16), ("bgr", 8)):
                w_sb[nm] = wp.tile([P, hw], F32, name=nm + "_sb")
                nc.sync.dma_start(out=w_sb[nm][:], in_=T[nm][:])
            bf8_sb = wp.tile([1, 4 * E], F8, name="bf8_sb")
            nc.sync.dma_start(out=bf8_sb[:], in_=T["bf8"][:])

            def gdma(idx, t):
                raw = rp.tile([P, E], F8, tag="raw", bufs=6, name="raw")
                nc.gpsimd.indirect_dma_start(
                    out=raw[:], out_offset=None, in_=T["emb8"][:],
                    in_offset=bass.IndirectOffsetOnAxis(
                        ap=idxs[idx][:, t:t + 1], axis=0))
                return raw

            def gfin(raw, dstT8, t, ceng):
                # fp8 transpose outputs must land with element step 2
                ptile = ptp.tile([P, NE, P, 2], F8, tag="pt", bufs=2,
                                 name="ptile")
                for et in range(NE):
                    nc.tensor.transpose(
                        out=ptile[:, et, :, 0],
                        in_=raw[:, et * P:(et + 1) * P], identity=ident8[:])
                if ceng is nc.vector:
                    nc.vector.tensor_copy(
                        out=dstT8[:, :, t * P:(t + 1) * P],
                        in_=ptile[:, :, :, 0])
                else:  # Act engine copy (GpSimd cannot read PSUM)
                    nc.scalar.activation(
                        out=dstT8[:, :, t * P:(t + 1) * P],
                        in_=ptile[:, :, :, 0], func=AF.Identity)

            e0T8 = ap_.tile([P, NE, NL], F8)
            xT8 = ap_.tile([P, NE, NL], F8)
            e1T8 = ap_.tile([P, NE, NL], F8)
            for t in range(NL // P):
                gfin(gdma("idx_x", t), xT8, t, nc.vector)
            for t in range(NL // P):
                gfin(gdma("idx_e0", t), e0T8, t, nc.vector)
            e1raw = [gdma("idx_e1", t) for t in range(NL // P)]

            # ---- PE gate streams (priority order: f, c1, then c2/rcells)
            fps = [_xgates(nc, pgp, xT8, w_sb["wf"], ZG, NE, s0=2 * h)
                   for h in range(2)]
            c1ps = _xgates(nc, pgp, e0T8, w_sb["wgf"], ZG, NH)

            def zcell(ps, b_sb, hc, s0, h_out, c_out, prebias=False):
                """Zero-state cell tail (bf16): acts (+bias), c/h chain."""
                gb = {}
                for g, func in ((0, AF.Sigmoid), (2, AF.Tanh),
                                (3, AF.Sigmoid)):
                    t = gp.tile([P, 2, NL], BF16, tag=f"zg{g}", bufs=2,
                                name=f"zg{g}")
                    if prebias:
                        nc.scalar.activation(out=t[:], in_=ps[g][:],
                                             func=func)
                    else:
                        for s in range(2):
                            jc = g * hc + s0 + s
                            nc.scalar.activation(
                                out=t[:, s, :], in_=ps[g][:, s, :],
                                func=func, bias=b_sb[:, jc:jc + 1])
                    gb[g] = t
                nc.vector.tensor_mul(c_out[:], gb[0][:], gb[2][:])
                tc_ = tp.tile([P, 2, NL], BF16, tag="t2", bufs=5,
                              name="tanhc")
                nc.scalar.activation(out=tc_[:], in_=c_out[:], func=AF.Tanh)
                nc.vector.tensor_mul(h_out[:], gb[3][:], tc_[:])

            def half_T(src8, ets, row0, col0):
                """Transpose an e-major g half to n-major; one send DMA to
                the AG2 src rows [row0, row0+NL) cols [col0, col0+G)."""
                with tc.high_priority():
                    _half_T(src8, ets, row0, col0)

            def _half_T(src8, ets, row0, col0):
                htile = tp.tile([P, NL // P, G], F8, tag="htile", bufs=2,
                                name="htile")
                for nt in range(NL // P):
                    ptile = ptp.tile([P, NE, P, 2], F8, tag="pt", bufs=2,
                                     name="pth")
                    for j in range(NH):
                        nc.tensor.transpose(
                            out=ptile[:, j, :, 0],
                            in_=src8[:, ets + j, nt * P:(nt + 1) * P],
                            identity=ident8[:])
                    nc.vector.tensor_copy(
                        out=htile[:, nt, :].rearrange("p (j q) -> p j q",
                                                      q=P),
                        in_=ptile[:, 0:NH, :, 0])
                nc.sync.dma_start(
                    out=ag2s[row0:row0 + NL, col0:col0 + G].rearrange(
                        "(c p) g -> p c g", p=P),
                    in_=htile[:])

            cfT = ap_.tile([P, NH, NL], BF16, name="cfT")
            crT = ap_.tile([P, NH, NL], BF16, name="crT")

            # f-cell -> fh8 = h + x, ship AG1 as early as possible
            fh8 = ap_.tile([P, NE, NL], F8, name="fh8")
            for h in range(2):
                hs_ = slice(2 * h, 2 * h + 2)
                cf_ = tp.tile([P, 2, NL], BF16, tag="t2", bufs=5, name="cf")
                hf_ = tp.tile([P, 2, NL], BF16, tag="t2", bufs=5, name="hf")
                zcell(fps[h], w_sb["bf"], NE, 2 * h, hf_, cf_)
                nc.vector.tensor_add(fh8[:, hs_, :], hf_[:], xT8[:, hs_, :])
            with tc.high_priority():
                nc.sync.dma_start(
                    out=ag1s[:].rearrange("(et p) n -> p et n", p=P),
                    in_=fh8[:])
                nc.gpsimd.collective_compute(
                    "AllGather", ALU.bypass, replica_groups=rg,
                    ins=[ag1_src_w[:].opt()], outs=[ag1_dst_w[:].opt()])

            # e1 copies slot into the DVE gap between the f and c1 tails
            for t in range(NL // P):
                gfin(e1raw[t], e1T8, t, nc.vector)

            zcell(c1ps, w_sb["bgf"], NH, 0, g08[:, 0:NH, :], cfT)    # hf0
            c2ps = _xgates(nc, pgp, e1T8, w_sb["wgr"], ZG, NH)
            half_T(g08, 0, 0, 0)                                     # hf0
            zcell(c2ps, w_sb["bgr"], NH, 0, g18[:, NH:NE, :], crT)   # hr1
            half_T(g18, NH, NL, G)                                   # hr1

            def rcell(xT8c, W_sb, U_sb, b_sb, hprev8, cprev, h_out):
                """Recurrent cell: x-gates + U@h accumulate in PSUM, acts
                read PSUM with bias; bf16 c-chain on DVE."""
                gb = []
                for g in range(4):
                    ups = pgp.tile([P, 2, NL], F32, tag="pg2", bufs=3,
                                   name="ups")
                    for s in range(2):
                        jc = g * NH + s
                        js = slice(jc * P, (jc + 1) * P)
                        for i in range(NE // 2):
                            nc.tensor.matmul(
                                ups[:, s, :], W_sb[:, 2 * i:2 * i + 2, js],
                                xT8c[:, 2 * i:2 * i + 2, :],
                                start=(i == 0), stop=False,
                                perf_mode=DR, skip_group_check=True)
                        nc.tensor.matmul(
                            ups[:, s, :], U_sb[:, :, js],
                            hprev8[:], start=False, stop=True, perf_mode=DR,
                            skip_group_check=True)
                    gt = gp.tile([P, 2, NL], BF16, tag=f"rg{g}", bufs=2,
                                 name=f"rg{g}")
                    func = AF.Tanh if g == 2 else AF.Sigmoid
                    for s in range(2):
                        nc.scalar.activation(
                            out=gt[:, s, :], in_=ups[:, s, :], func=func,
                            bias=b_sb[:, g * NH + s:g * NH + s + 1])
                    gb.append(gt)
                ig = tp.tile([P, 2, NL], BF16, tag="t2", bufs=5, name="ig")
                nc.vector.tensor_mul(ig[:], gb[0][:], gb[2][:])
                cc = tp.tile([P, 2, NL], BF16, tag="t2", bufs=5, name="cc")
                nc.vector.tensor_mul(cc[:], gb[1][:], cprev[:])
                nc.vector.tensor_add(cc[:], cc[:], ig[:])
                tc_ = tp.tile([P, 2, NL], BF16, tag="t2", bufs=5,
                              name="tanhc")
                nc.scalar.activation(out=tc_[:], in_=cc[:], func=AF.Tanh)
                nc.vector.tensor_mul(h_out[:], gb[3][:], tc_[:])

            rcell(e1T8, w_sb["wgf"], w_sb["ugf"], w_sb["bgf"],
                  g08[:, 0:NH, :], cfT, g18[:, 0:NH, :])   # hf1
            half_T(g18, 0, NL, 0)                                    # hf1
            rcell(e0T8, w_sb["wgr"], w_sb["ugr"], w_sb["bgr"],
                  g18[:, NH:NE, :], crT, g08[:, NH:NE, :])  # hr0
            half_T(g08, NH, 0, G)                                    # hr0
            with tc.high_priority():
                nc.gpsimd.collective_compute(
                    "AllGather", ALU.bypass, replica_groups=rg,
                    ins=[ag2_src_w[:].opt()], outs=[ag2_dst_w[:].opt()])

            nc.vector.tensor_sub(dgT8[:], g08[:], g18[:])
            # S1 = local colsum of g1 (row 7)
            for et in range(NE):
                nc.vector.reduce_sum(out=stag[:, 7, et:et + 1],
                                     in_=g18[:, et, :],
                                     axis=mybir.AxisListType.X)

            # k-chunked preloads on the gpsimd queue, emitted after both
            # collective triggers so they cannot block them
            for k in range(NCORES):
                nc.gpsimd.dma_start(
                    out=fhAll[:, k, :, :],
                    in_=ag1d[k * E:(k + 1) * E, :].rearrange(
                        "(et p) n -> p et n", p=P))
            for k in range(NCORES):
                nc.gpsimd.dma_start(
                    out=gAll[:, 8 * k:8 * k + 8, :],
                    in_=ag2d[k * 2 * NL:(k + 1) * 2 * NL, :].rearrange(
                        "(b c p) e -> p (b c) e", p=P, b=2))

        # ---- phase D: D1 (a0) per k-block, then rq, then reductions ----
        with tc.tile_pool(name="pp", bufs=1, space="PSUM") as pp, \
             tc.tile_pool(name="fin", bufs=1) as fin:
            for k in range(NCORES):
                for cp in range(2):
                    pd2 = pp.tile([P, 2, NL], F32, tag="pd2", bufs=4,
                                  name="pd2")
                    for cc in range(2):
                        c = 2 * cp + cc
                        for i in range(NE // 2):
                            nc.tensor.matmul(
                                pd2[:, cc, :],
                                fhAll[:, k, 2 * i:2 * i + 2,
                                      c * P:(c + 1) * P],
                                dgT8[:, 2 * i:2 * i + 2, :],
                                start=(i == 0), stop=(i == NE // 2 - 1),
                                perf_mode=DR)
                    mb = 4 * k + 2 * cp
                    nc.scalar.activation(
                        out=A0T[:, mb:mb + 2, :], in_=pd2[:], func=AF.Sigmoid)

            # sg0/sg1 (rows 2,3) via Act Square+accum in the pre-D1 gap
            for row, gT in ((2, g08), (3, g18)):
                for et in range(NE):
                    junk = fin.tile([P, NL], BF16, tag="junk", bufs=4,
                                    name="junk")
                    nc.scalar.activation(out=junk[:], in_=gT[:, et, :],
                                         func=AF.Square,
                                         accum_out=stag[:, row, et:et + 1])

            def ered(r_t, q_t, ets_r, ets_q):
                """Phase-E reductions for one rq half: r rows into stag
                0/1, q rows 4/5/6."""
                for j in range(2):
                    et = ets_r[j]
                    scr2 = fin.tile([P, NL], F32, tag="scr2", bufs=2,
                                    name="scr2")
                    nc.vector.tensor_mul(scr2[:], r_t[:, j, :],
                                         g08[:, et, :])
                    nc.vector.reduce_sum(out=stag[:, 0, et:et + 1],
                                         in_=scr2[:],
                                         axis=mybir.AxisListType.X)
                    junk = fin.tile([P, NL], F32, tag="junk2", bufs=4,
                                    name="junkr")
                    nc.scalar.activation(out=junk[:], in_=r_t[:, j, :],
                                         func=AF.Square,
                                         accum_out=stag[:, 1, et:et + 1])
                for j in range(2):
                    et = ets_q[j]
                    scr3 = fin.tile([P, NL], F32, tag="scr2", bufs=2,
                                    name="scr3")
                    nc.vector.tensor_mul(scr3[:], q_t[:, j, :],
                                         g18[:, et, :])
                    nc.vector.reduce_sum(out=stag[:, 6, et:et + 1],
                                         in_=scr3[:],
                                         axis=mybir.AxisListType.X)
                    junk2 = fin.tile([P, NL], F32, tag="junk2", bufs=4,
                                     name="junkq")
                    nc.scalar.activation(out=junk2[:], in_=q_t[:, j, :],
                                         func=AF.Square,
                                         accum_out=stag[:, 5, et:et + 1])
                    junk3 = fin.tile([P, NL], F32, tag="junk2", bufs=4,
                                     name="junkq2")
                    nc.scalar.activation(out=junk3[:], in_=q_t[:, j, :],
                                         func=AF.Identity,
                                         accum_out=stag[:, 4, et:et + 1])

            def rqpass(boff, banks):
                """One rq tensor pass (g0: boff=0, g1: boff=4), k-outer so
                gAll chunks are consumed as they land."""
                for k in range(NCORES):
                    for cp in range(2):
                        a0sl = A0T[:, 4 * k + 2 * cp:4 * k + 2 * cp + 2, :]
                        st = (k == 0 and cp == 0)
                        sp = (k == NCORES - 1 and cp == 1)
                        gb = slice(8 * k + boff + 2 * cp,
                                   8 * k + boff + 2 * cp + 2)
                        for dst, j, es in banks:
                            nc.tensor.matmul(
                                dst[:, j, :],
                                gAll[:, gb, es * P:(es + 1) * P], a0sl,
                                start=st, stop=sp, perf_mode=DR)

            r0p = [pp.tile([P, 2, NL], F32, tag="pd2", bufs=4,
                           name=f"r0{j}") for j in range(2)]
            rqpass(0, [(r0p[0], 0, 0), (r0p[0], 1, 1),
                       (r0p[1], 0, 2), (r0p[1], 1, 3)])
            # dot0 / sr0 reductions overlap the q1 pass
            for et in range(NE):
                r_t = r0p[et // 2][:, et % 2, :]
                scr2 = fin.tile([P, NL], F32, tag="scr2", bufs=2,
                                name="scr2")
                nc.vector.tensor_mul(scr2[:], r_t, g08[:, et, :])
                nc.vector.reduce_sum(out=stag[:, 0, et:et + 1], in_=scr2[:],
                                     axis=mybir.AxisListType.X)
                junk = fin.tile([P, NL], F32, tag="junk2", bufs=4,
                                name="junkr")
                nc.scalar.activation(out=junk[:], in_=r_t, func=AF.Square,
                                     accum_out=stag[:, 1, et:et + 1])

            q1p = [pp.tile([P, 2, NL], F32, tag="pd2", bufs=4,
                           name=f"q1{j}") for j in range(2)]
            rqpass(4, [(q1p[0], 0, 0), (q1p[0], 1, 1),
                       (q1p[1], 0, 2), (q1p[1], 1, 3)])
            for et in range(NE):
                q_t = q1p[et // 2][:, et % 2, :]
                scr3 = fin.tile([P, NL], F32, tag="scr2", bufs=2,
                                name="scr3")
                nc.vector.tensor_mul(scr3[:], q_t, g18[:, et, :])
                nc.vector.reduce_sum(out=stag[:, 6, et:et + 1], in_=scr3[:],
                                     axis=mybir.AxisListType.X)
                junk2 = fin.tile([P, NL], F32, tag="junk2", bufs=4,
                                 name="junkq")
                nc.scalar.activation(out=junk2[:], in_=q_t, func=AF.Square,
                                     accum_out=stag[:, 5, et:et + 1])
                junk3 = fin.tile([P, NL], F32, tag="junk2", bufs=4,
                                 name="junkq2")
                nc.scalar.activation(out=junk3[:], in_=q_t,
                                     func=AF.Identity,
                                     accum_out=stag[:, 4, et:et + 1])

        # single transposed output DMA: stag [P, 8, 4] -> out [8, E]
        with tc.tile_pool(name="po", bufs=1, space="PSUM") as pop, \
             tc.tile_pool(name="fo", bufs=1) as fop:
            ot = pop.tile([32, P], F32)
            nc.tensor.transpose(out=ot[:],
                                in_=stag[:].rearrange("p r e -> p (r e)"),
                                identity=identf[:])
            os_ = fop.tile([32, P], F32)
            nc.vector.tensor_copy(out=os_[:], in_=ot[:])
            nc.sync.dma_start(
                out=T["out"][:].rearrange("r (et p) -> (r et) p", p=P),
                in_=os_[:])


_PROGRAM = None


def _get_program():
    global _PROGRAM
    if _PROGRAM is None:
        _PROGRAM = build_program()
    return _PROGRAM


def _f8np():
    return mybir.dt.np(F8)


def _prep_w(w):
    """(4H, E_in) torch-layout weight -> fp8 lhsT tiles [p, kt, 4H]."""
    wt = np.asarray(w, np.float32).T  # (E_in, 4H)
    e_in, fourh = wt.shape
    t = np.ascontiguousarray(
        wt.reshape(e_in // P, P, fourh).transpose(1, 0, 2))
    return t.astype(_f8np())


def _prep_b(b1, b2):
    """Summed bias laid out [P, n_chunks] f32 (per-chunk activation bias)."""
    s = np.asarray(b1, np.float32) + np.asarray(b2, np.float32)
    return np.ascontiguousarray(s.reshape(-1, P).T)


def run_device(inputs, trace=False):
    """Shard inputs, run the 8-core SPMD program, return results."""
    nc = _get_program()
    emb8 = np.asarray(inputs["embedding"], np.float32).astype(_f8np())
    iq = np.asarray(inputs["input"]).astype(np.int32).reshape(N, 1)
    ie = np.asarray(inputs["set_inputs"]).astype(np.int32)
    shared = {
        "emb8": np.ascontiguousarray(emb8),
        "wgf": _prep_w(inputs["wih_gf"]), "wgr": _prep_w(inputs["wih_gr"]),
        "ugf": _prep_w(inputs["whh_gf"]), "ugr": _prep_w(inputs["whh_gr"]),
        "wf": _prep_w(inputs["wih_f"]),
        "bgf": _prep_b(inputs["bih_gf"], inputs["bhh_gf"]),
        "bgr": _prep_b(inputs["bih_gr"], inputs["bhh_gr"]),
        "bf": _prep_b(inputs["bih_f"], inputs["bhh_f"]),
        "bf8": (np.asarray(inputs["bih_f"], np.float32)
                + np.asarray(inputs["bhh_f"], np.float32)
                ).reshape(1, -1).astype(_f8np()),
    }
    in_maps = []
    for k in range(NCORES):
        sl = slice(k * NL, (k + 1) * NL)
        m = dict(shared)
        m["idx_x"] = np.ascontiguousarray(iq[sl])
        m["idx_e0"] = np.ascontiguousarray(ie[0, sl].reshape(NL, 1))
        m["idx_e1"] = np.ascontiguousarray(ie[1, sl].reshape(NL, 1))
        in_maps.append(m)
    res = bass_utils.run_bass_kernel_spmd(
        nc, in_maps, core_ids=list(range(NCORES)), trace=trace)
    return res


def kernel(**inputs):
    res = run_device(inputs)
    return host_tail(res, inputs)


def host_tail(res, inputs):
    acc = np.zeros((8, E), np.float64)
    for r in res.results:
        acc += r["out"]
    dot0, sr0, sg0, sg1, A, B, C, S1 = acc
    dot1 = S1 * S1 - C
    sr1 = N * S1 * S1 - 2.0 * S1 * A + B
    dot = np.stack([dot0, dot1])
    sr = np.stack([sr0, sr1])
    sg = np.stack([sg0, sg1])
    nr = np.maximum(np.sqrt(sr), EPS)
    ng = np.maximum(np.sqrt(sg), EPS)
    cos = dot / (nr * ng)
    kern = cos / np.exp(cos).sum()
    w_out = np.asarray(inputs["w_out"], np.float64)
    b_out = np.asarray(inputs["b_out"], np.float64)
    k2 = kern @ w_out.T + b_out                  # (2, R)
    s = k2.sum(axis=1)                           # (2,)
    labels = np.asarray(inputs["set_labels"], np.float64)
    o = s[0] * labels[0] + s[1] * labels[1]      # (R,)
    o = np.exp(o - o.max())
    o /= o.sum()
    return o.astype(np.float32)
